# revision 14
# baseline (speedup 1.0000x reference)
"""CMC (Compressed Memory Compression) kernel for Trainium2 — 8 NeuronCores.

Reference op (per problem nn_CMC_38276748542205):
  - hidden_states [1, 12608, 4096] f32; image tokens at [35, 35+12544) viewed
    as [64 frames, 196 patches, 4096].
  - Frames form 16 intervals of 4; I-frame at position 3 of each interval.
  - SAD(token, I-frame token at same patch) over dim; mask = SAD < 1.12*4096.
  - Masked tokens replaced by the interval's I-frame token.

Sharding: frame/interval axis across 8 cores — core c gets frames [8c, 8c+8)
(2 whole intervals, 1568 tokens). Text tokens (64 rows) pass through on host.

Device kernel (per core, SPMD) — sparse-scatter formulation:
  The replacement value for a masked token is bit-exactly the interval's
  I-frame token, and unmasked tokens are bit-exactly the input — so a full
  write-back (25.7 MB/core) is wasted HBM traffic.  Instead:
  - three full-width super-chunks per core: A0 = iv0 patches 0-127,
    A1 = iv1 patches 0-127, Bp = patches 128-191 of BOTH intervals packed
    on the partition axis (iv0 -> rows 0-63, iv1 -> rows 64-127), so every
    DVE/ACT pass runs all 128 lanes (12 -> 9 elementwise units);
  - per unit: DVE d = p3 - p_k; ACT |d| in-place with per-2048-chunk
    accumulation -> SAD (chunked so fp32 summation error stays below the
    min |SAD-thr| margin of ~0.034 — the 2e-2 rel-err budget only allows
    ~2 flipped tokens, so the SAD math must stay exact-fp32);
  - DVE: m = (sad < thr) per-partition 0/1 into a [128, 9] mask tile,
    stored once at the end (4.6 KB) for the host merge;
  - scatter index: idx = iota_col*m + BIG; iota_col encodes the target
    patch row (and 0 for rows belonging to the other interval, so idx=BIG
    there); gpsimd indirect DMA with bounds_check silently skips OOB rows,
    writing ONLY the ~26% replaced rows (the f3 slice) into that
    (iv,k,chunk)'s own output tensor.  Disjoint write sets per tensor keep
    the tile scheduler from chaining the scatters on a conservative WAW
    hazard; indirect DMA APs must start at partition 0 (non-zero start
    wedges the device), so per-iv scatters from the packed tile use a
    full-width index window with the other half forced OOB.
  Device traffic: ~24 MB loads + ~5 MB scatter vs 51.4 MB baseline. Host
  merge: out starts as a copy of the input; only mask-selected rows are
  copied from the scatter outputs (everything else is already correct by
  identity).
"""

import functools

import numpy as np

# ---- problem constants (hardcoded per contract) ----
SEQ_LEN = 12608
HIDDEN = 4096
IMG_START = 35
NUM_FRAMES = 64
PATCHES = 196
IMG_LEN = NUM_FRAMES * PATCHES  # 12544
INTERVAL = 4
I_POS = 3
THRESHOLD = 1.12 * HIDDEN  # 4587.52

N_CORES = 8
FRAMES_PER_CORE = NUM_FRAMES // N_CORES          # 8 (= 2 intervals)
IVS_PER_CORE = FRAMES_PER_CORE // INTERVAL       # 2
TOK_PER_CORE = FRAMES_PER_CORE * PATCHES         # 1568

SAD_CHUNK = 2048       # accumulation chunk for SAD numerical accuracy
BIG = 4096.0           # OOB scatter index (> PATCHES-1 -> skipped)
N_MASK_COLS = 9        # A0 k=0..2 | A1 k=0..2 | Bp k=0..2
N_YS = IVS_PER_CORE * 3 * 2  # 12 scatter-target tensors: (iv, k, chunkA/B)


def _kernel_body(tc, ys_aps, ym_ap, x_ap):
    import concourse.bass as bass
    from concourse import mybir

    nc = tc.nc
    AF = mybir.ActivationFunctionType
    OP = mybir.AluOpType
    f32 = mybir.dt.float32
    i32 = mybir.dt.int32

    xv = x_ap.rearrange("(f p) d -> p f d", f=FRAMES_PER_CORE, p=PATCHES)

    import contextlib

    with contextlib.ExitStack() as ctx:
        pI_pool = ctx.enter_context(tc.tile_pool(name="pI", bufs=3))
        pP_pool = ctx.enter_context(tc.tile_pool(name="pP", bufs=2))
        d_pool = ctx.enter_context(tc.tile_pool(name="d", bufs=2))
        small_pool = ctx.enter_context(tc.tile_pool(name="small", bufs=12))
        hold_pool = ctx.enter_context(tc.tile_pool(name="hold", bufs=1))

        n_sad_chunks = HIDDEN // SAD_CHUNK

        # one-time tiles.  iota_c columns hold per-scatter index bases
        # (patch - BIG, or 0 on rows that must stay OOB):
        #   col 0: A rows     -> p - BIG            (full 128)
        #   col 1: Bp iv0     -> 128+p - BIG on [0:64), 0 elsewhere
        #   col 2: Bp iv1     -> 64+p - BIG on [64:128), 0 elsewhere
        iota_i = hold_pool.tile([128, 1], i32, tag="iotai")
        nc.gpsimd.iota(iota_i[:, :], [[0, 1]], base=0, channel_multiplier=1)
        iota_f = hold_pool.tile([128, 1], f32, tag="iotaf")
        nc.vector.tensor_copy(iota_f[:, :], iota_i[:, :])
        iota_c = hold_pool.tile([128, 3], f32, tag="iotac")
        nc.vector.tensor_scalar(
            iota_c[:, 0:1], iota_f[:, :], float(-BIG), None, op0=OP.add
        )
        nc.vector.tensor_scalar(
            iota_c[:, 1:2], iota_f[:, :], float(128.0 - BIG), None, op0=OP.add
        )
        nc.vector.memset(iota_c[64:128, 1:2], 0.0)
        nc.vector.tensor_scalar(
            iota_c[:, 2:3], iota_f[:, :], float(64.0 - BIG), None, op0=OP.add
        )
        nc.vector.memset(iota_c[0:64, 2:3], 0.0)
        mask_sb = hold_pool.tile([128, N_MASK_COLS], f32, tag="mask")

        # Scatter spec per (super-chunk, k): list of (ys index, iota col).
        # A chunks write one tensor; the packed-B chunk does one scatter per
        # interval, the other interval's rows forced OOB via the iota col.
        def scatter_specs(sc, k):
            if sc < 2:  # A0 / A1
                return [(sc * 6 + k * 2 + 0, 0)]
            return [(0 * 6 + k * 2 + 1, 1), (1 * 6 + k * 2 + 1, 2)]

        def compute_k(pt3, ptk, sc, k):
            d_t = d_pool.tile([128, HIDDEN], f32)
            nc.vector.tensor_tensor(
                d_t[:, :], pt3[:, :], ptk[:, :], op=OP.subtract
            )
            sadp = small_pool.tile([128, n_sad_chunks], f32, tag="sadp")
            for h in range(n_sad_chunks):
                # |d| in place (out aliases in); only accum_out is consumed
                nc.scalar.activation(
                    d_t[:, bass.ts(h, SAD_CHUNK)],
                    d_t[:, bass.ts(h, SAD_CHUNK)],
                    AF.Abs,
                    accum_out=sadp[:, h : h + 1],
                )
            m_col = mask_sb[:, sc * 3 + k : sc * 3 + k + 1]
            # fused: m = (sadp0 + sadp1) < thr — both scalars per-partition
            nc.vector.tensor_scalar(
                m_col[:, :],
                sadp[:, 0:1],
                sadp[:, 1:2],
                float(THRESHOLD),
                op0=OP.add,
                op1=OP.is_lt,
            )
            for ys_idx, icol in scatter_specs(sc, k):
                # idx = iota_col*m + BIG: masked -> patch row, else OOB
                idx_f = small_pool.tile([128, 1], f32, tag="idxf")
                nc.vector.tensor_scalar(
                    idx_f[:, :],
                    iota_c[:, icol : icol + 1],
                    m_col[:, 0:1],
                    BIG,
                    op0=OP.mult,
                    op1=OP.add,
                )
                idx_i = small_pool.tile([128, 1], i32, tag="idxi")
                nc.vector.tensor_copy(idx_i[:, :], idx_f[:, :])
                nc.gpsimd.indirect_dma_start(
                    out=ys_aps[ys_idx],
                    out_offset=bass.IndirectOffsetOnAxis(
                        ap=idx_i[:, 0:1], axis=0
                    ),
                    in_=pt3[:, :],
                    in_offset=None,
                    bounds_check=PATCHES - 1,
                    oob_is_err=False,
                )

        ld = [0]

        def load(dst, src):
            # alternate the two HWDGE rings so streams interleave
            eng = nc.sync if ld[0] % 2 == 0 else nc.scalar
            ld[0] += 1
            eng.dma_start(dst, src)

        # super-chunks: ("A", iv) = patches 0-127 of one interval at full
        # width; ("B",) = patches 128-191 of BOTH intervals packed on the
        # partition axis (iv0 -> rows [0:64), iv1 -> rows [64:128))
        for sc, kind in enumerate(("A0", "B", "A1")):
            ptI = pI_pool.tile([128, 2, HIDDEN], f32, tag="ptI")
            ptP = pP_pool.tile([128, 2, HIDDEN], f32, tag="ptP")
            if kind == "B":
                for ivx in range(IVS_PER_CORE):
                    f0 = ivx * INTERVAL
                    rows = slice(ivx * 64, ivx * 64 + 64)
                    load(ptI[rows, :, :], xv[128:192, f0 + 2 : f0 + 4, :])
                    load(ptP[rows, :, :], xv[128:192, f0 : f0 + 2, :])
                scn = 2
            else:
                f0 = (0 if kind == "A0" else 1) * INTERVAL
                load(ptI[:, :, :], xv[0:128, f0 + 2 : f0 + 4, :])
                load(ptP[:, :, :], xv[0:128, f0 : f0 + 2, :])
                scn = 0 if kind == "A0" else 1
            for k in (2, 0, 1):  # f=3 (I-frame) never changes
                ptk = ptI[:, 0, :] if k == 2 else ptP[:, k, :]
                compute_k(ptI[:, 1, :], ptk, scn, k)

        # single 4.6 KB mask store for the host merge
        nc.sync.dma_start(ym_ap, mask_sb[:, :])


@functools.cache
def _build_nc():
    import concourse.bacc as bacc
    import concourse.tile as tile
    from concourse import mybir

    nc = bacc.Bacc(
        "TRN2",
        target_bir_lowering=False,
        debug=False,
        enable_asserts=False,
        num_devices=N_CORES,
    )
    x = nc.dram_tensor(
        "x", [TOK_PER_CORE, HIDDEN], mybir.dt.float32, kind="ExternalInput"
    ).ap()
    ys = [
        nc.dram_tensor(
            f"ys{c}", [PATCHES, HIDDEN], mybir.dt.float32, kind="ExternalOutput"
        ).ap()
        for c in range(N_YS)
    ]
    ym = nc.dram_tensor(
        "ym", [128, N_MASK_COLS], mybir.dt.float32, kind="ExternalOutput"
    ).ap()
    with tile.TileContext(nc) as tc:
        _kernel_body(tc, ys, ym, x)
    nc.compile()
    return nc


def _in_maps(hs: np.ndarray):
    img = hs[0, IMG_START : IMG_START + IMG_LEN]
    maps = []
    for c in range(N_CORES):
        xc = img[TOK_PER_CORE * c : TOK_PER_CORE * (c + 1)]
        maps.append({"x": np.ascontiguousarray(xc)})
    return maps


def _host_runt(img: np.ndarray) -> np.ndarray:
    """Blend for patches 192-195 (the %16 runt the device skips): numpy."""
    iv = img.reshape(NUM_FRAMES // INTERVAL, INTERVAL, PATCHES, HIDDEN)
    runt = iv[:, :, 192:PATCHES, :]
    itok = runt[:, I_POS : I_POS + 1]
    d = itok.astype(np.float64) - runt.astype(np.float64)
    mask = np.abs(d).sum(-1) < THRESHOLD
    return np.where(mask[..., None], itok, runt).astype(np.float32)


def _mask_cells(ym: np.ndarray):
    """Decode the [128, 9] mask tile.

    Yields (iv, k, ys_idx, patches, part_rows): masked patch numbers and the
    partition rows they came from, per (interval, P-frame) scatter tensor.
    """
    for sc in range(3):
        for k in range(3):
            col = ym[:, sc * 3 + k]
            if sc < 2:  # A0 / A1: partition p == patch p
                sel = np.nonzero(col[0:128] > 0.5)[0]
                yield sc, k, sc * 6 + k * 2 + 0, sel, sel
            else:  # packed B: rows [0:64) iv0, [64:128) iv1; patch 128+r
                for ivx in range(IVS_PER_CORE):
                    r = np.nonzero(col[ivx * 64 : ivx * 64 + 64] > 0.5)[0]
                    yield ivx, k, ivx * 6 + k * 2 + 1, 128 + r, ivx * 64 + r


def kernel(hidden_states: np.ndarray) -> np.ndarray:
    from concourse.bass_utils import run_bass_kernel_spmd

    hs = np.asarray(hidden_states, dtype=np.float32)
    assert hs.shape == (1, SEQ_LEN, HIDDEN), hs.shape
    nc = _build_nc()
    res = run_bass_kernel_spmd(nc, _in_maps(hs), list(range(N_CORES)))
    out = hs.copy()
    img_flat = out[0, IMG_START : IMG_START + IMG_LEN]  # [12544, 4096] view
    for c in range(N_CORES):
        ym = np.asarray(res.results[c]["ym"])   # [128, 9] 0/1 mask columns
        for ivx, k, ys_idx, patches, _rows in _mask_cells(ym):
            if patches.size:
                ys = np.asarray(res.results[c][f"ys{ys_idx}"])
                r = (ivx * INTERVAL + k) * PATCHES
                img_flat[TOK_PER_CORE * c + r + patches] = ys[patches]
    img = hs[0, IMG_START : IMG_START + IMG_LEN]
    outv = img_flat.reshape(NUM_FRAMES, PATCHES, HIDDEN)
    outv[:, 192:PATCHES, :] = _host_runt(img).reshape(NUM_FRAMES, 4, HIDDEN)
    return out


# revision 15
# speedup vs baseline: 1.0960x; 1.0960x over previous
"""CMC (Compressed Memory Compression) kernel for Trainium2 — 8 NeuronCores.

Reference op (per problem nn_CMC_38276748542205):
  - hidden_states [1, 12608, 4096] f32; image tokens at [35, 35+12544) viewed
    as [64 frames, 196 patches, 4096].
  - Frames form 16 intervals of 4; I-frame at position 3 of each interval.
  - SAD(token, I-frame token at same patch) over dim; mask = SAD < 1.12*4096.
  - Masked tokens replaced by the interval's I-frame token.

Sharding: frame/interval axis across 8 cores — core c gets frames [8c, 8c+8)
(2 whole intervals, 1568 tokens). Text tokens (64 rows) pass through on host.

Device kernel (per core, SPMD) — sparse-scatter formulation:
  The replacement value for a masked token is bit-exactly the interval's
  I-frame token, and unmasked tokens are bit-exactly the input — so a full
  write-back (25.7 MB/core) is wasted HBM traffic.  Instead:
  - three full-width super-chunks per core: A0 = iv0 patches 0-127,
    A1 = iv1 patches 0-127, Bp = patches 128-191 of BOTH intervals packed
    on the partition axis (iv0 -> rows 0-63, iv1 -> rows 64-127), so every
    DVE/ACT pass runs all 128 lanes (12 -> 9 elementwise units);
  - per unit: DVE d = p3 - p_k; ACT |d| in-place with per-2048-chunk
    accumulation -> SAD (chunked so fp32 summation error stays below the
    min |SAD-thr| margin of ~0.034 — the 2e-2 rel-err budget only allows
    ~2 flipped tokens, so the SAD math must stay exact-fp32);
  - DVE: m = (sad < thr) per-partition 0/1 into a [128, 9] mask tile,
    stored once at the end (4.6 KB) for the host merge;
  - scatter index: idx = iota_col*m + BIG; iota_col encodes the target
    patch row (and 0 for rows belonging to the other interval, so idx=BIG
    there); gpsimd indirect DMA with bounds_check silently skips OOB rows,
    writing ONLY the ~26% replaced rows (the f3 slice) into that
    (iv,k,chunk)'s own output tensor.  Disjoint write sets per tensor keep
    the tile scheduler from chaining the scatters on a conservative WAW
    hazard; indirect DMA APs must start at partition 0 (non-zero start
    wedges the device), so per-iv scatters from the packed tile use a
    full-width index window with the other half forced OOB.
  Device traffic: ~24 MB loads + ~5 MB scatter vs 51.4 MB baseline. Host
  merge: out starts as a copy of the input; only mask-selected rows are
  copied from the scatter outputs (everything else is already correct by
  identity).
"""

import functools

import numpy as np

# ---- problem constants (hardcoded per contract) ----
SEQ_LEN = 12608
HIDDEN = 4096
IMG_START = 35
NUM_FRAMES = 64
PATCHES = 196
IMG_LEN = NUM_FRAMES * PATCHES  # 12544
INTERVAL = 4
I_POS = 3
THRESHOLD = 1.12 * HIDDEN  # 4587.52

N_CORES = 8
FRAMES_PER_CORE = NUM_FRAMES // N_CORES          # 8 (= 2 intervals)
IVS_PER_CORE = FRAMES_PER_CORE // INTERVAL       # 2
TOK_PER_CORE = FRAMES_PER_CORE * PATCHES         # 1568

SAD_CHUNK = 2048       # accumulation chunk for SAD numerical accuracy
BIG = 4096.0           # OOB scatter index (> PATCHES-1 -> skipped)
N_MASK_COLS = 9        # A0 k=0..2 | A1 k=0..2 | Bp k=0..2
N_YS = IVS_PER_CORE * 3 * 2  # 12 scatter-target tensors: (iv, k, chunkA/B)


def _kernel_body(tc, ys_aps, ym_ap, x_ap):
    import concourse.bass as bass
    from concourse import mybir

    nc = tc.nc
    AF = mybir.ActivationFunctionType
    OP = mybir.AluOpType
    f32 = mybir.dt.float32
    i32 = mybir.dt.int32

    xv = x_ap.rearrange("(f p) d -> p f d", f=FRAMES_PER_CORE, p=PATCHES)

    import contextlib

    with contextlib.ExitStack() as ctx:
        pI_pool = ctx.enter_context(tc.tile_pool(name="pI", bufs=3))
        pP_pool = ctx.enter_context(tc.tile_pool(name="pP", bufs=2))
        d_pool = ctx.enter_context(tc.tile_pool(name="d", bufs=2))
        small_pool = ctx.enter_context(tc.tile_pool(name="small", bufs=12))
        hold_pool = ctx.enter_context(tc.tile_pool(name="hold", bufs=1))

        n_sad_chunks = HIDDEN // SAD_CHUNK

        # one-time tiles.  iota_c columns hold per-scatter index bases
        # (patch - BIG, or 0 on rows that must stay OOB):
        #   col 0: A rows     -> p - BIG            (full 128)
        #   col 1: Bp iv0     -> 128+p - BIG on [0:64), 0 elsewhere
        #   col 2: Bp iv1     -> 64+p - BIG on [64:128), 0 elsewhere
        iota_i = hold_pool.tile([128, 1], i32, tag="iotai")
        nc.gpsimd.iota(iota_i[:, :], [[0, 1]], base=0, channel_multiplier=1)
        iota_f = hold_pool.tile([128, 1], f32, tag="iotaf")
        nc.vector.tensor_copy(iota_f[:, :], iota_i[:, :])
        iota_c = hold_pool.tile([128, 3], f32, tag="iotac")
        nc.vector.tensor_scalar(
            iota_c[:, 0:1], iota_f[:, :], float(-BIG), None, op0=OP.add
        )
        nc.vector.tensor_scalar(
            iota_c[:, 1:2], iota_f[:, :], float(128.0 - BIG), None, op0=OP.add
        )
        nc.vector.memset(iota_c[64:128, 1:2], 0.0)
        nc.vector.tensor_scalar(
            iota_c[:, 2:3], iota_f[:, :], float(64.0 - BIG), None, op0=OP.add
        )
        nc.vector.memset(iota_c[0:64, 2:3], 0.0)
        mask_sb = hold_pool.tile([128, N_MASK_COLS], f32, tag="mask")

        # Scatter spec per (super-chunk, k): list of (ys index, iota col).
        # A chunks write one tensor; the packed-B chunk does one scatter per
        # interval, the other interval's rows forced OOB via the iota col.
        def scatter_specs(sc, k):
            if sc < 2:  # A0 / A1
                return [(sc * 6 + k * 2 + 0, 0)]
            return [(0 * 6 + k * 2 + 1, 1), (1 * 6 + k * 2 + 1, 2)]

        def compute_k(pt3, ptk, sc, k):
            d_t = d_pool.tile([128, HIDDEN], f32)
            nc.vector.tensor_tensor(
                d_t[:, :], pt3[:, :], ptk[:, :], op=OP.subtract
            )
            sadp = small_pool.tile([128, n_sad_chunks], f32, tag="sadp")
            for h in range(n_sad_chunks):
                # |d| in place (out aliases in); only accum_out is consumed
                nc.scalar.activation(
                    d_t[:, bass.ts(h, SAD_CHUNK)],
                    d_t[:, bass.ts(h, SAD_CHUNK)],
                    AF.Abs,
                    accum_out=sadp[:, h : h + 1],
                )
            m_col = mask_sb[:, sc * 3 + k : sc * 3 + k + 1]
            # fused: m = (sadp0 + sadp1) < thr — both scalars per-partition
            nc.vector.tensor_scalar(
                m_col[:, :],
                sadp[:, 0:1],
                sadp[:, 1:2],
                float(THRESHOLD),
                op0=OP.add,
                op1=OP.is_lt,
            )
            for ys_idx, icol in scatter_specs(sc, k):
                # idx = iota_col*m + BIG: masked -> patch row, else OOB
                idx_f = small_pool.tile([128, 1], f32, tag="idxf")
                nc.vector.tensor_scalar(
                    idx_f[:, :],
                    iota_c[:, icol : icol + 1],
                    m_col[:, 0:1],
                    BIG,
                    op0=OP.mult,
                    op1=OP.add,
                )
                idx_i = small_pool.tile([128, 1], i32, tag="idxi")
                nc.vector.tensor_copy(idx_i[:, :], idx_f[:, :])
                nc.gpsimd.indirect_dma_start(
                    out=ys_aps[ys_idx],
                    out_offset=bass.IndirectOffsetOnAxis(
                        ap=idx_i[:, 0:1], axis=0
                    ),
                    in_=pt3[:, :],
                    in_offset=None,
                    bounds_check=PATCHES - 1,
                    oob_is_err=False,
                )

        ld = [0]

        def load(dst, src):
            # alternate the two HWDGE rings so streams interleave
            eng = nc.sync if ld[0] % 2 == 0 else nc.scalar
            ld[0] += 1
            eng.dma_start(dst, src)

        # super-chunks: ("A", iv) = patches 0-127 of one interval at full
        # width; ("B",) = patches 128-191 of BOTH intervals packed on the
        # partition axis (iv0 -> rows [0:64), iv1 -> rows [64:128))
        for kind in ("B", "A0", "A1"):
            ptI = pI_pool.tile([128, 2, HIDDEN], f32, tag="ptI")
            ptP = pP_pool.tile([128, 2, HIDDEN], f32, tag="ptP")
            if kind == "B":
                # rows [0:64) reach only even SBUF AXI ports and [64:128)
                # only odd ones — issue each pair back-to-back so the two
                # rings drive complementary port halves concurrently
                for half in (0, 1):  # 0: I-half (f2,f3), 1: P-half (f0,f1)
                    for ivx in range(IVS_PER_CORE):
                        f0 = ivx * INTERVAL + (2 if half == 0 else 0)
                        rows = slice(ivx * 64, ivx * 64 + 64)
                        dst = ptI if half == 0 else ptP
                        load(dst[rows, :, :], xv[128:192, f0 : f0 + 2, :])
                scn = 2
            else:
                f0 = (0 if kind == "A0" else 1) * INTERVAL
                load(ptI[:, :, :], xv[0:128, f0 + 2 : f0 + 4, :])
                load(ptP[:, :, :], xv[0:128, f0 : f0 + 2, :])
                scn = 0 if kind == "A0" else 1
            for k in (2, 0, 1):  # f=3 (I-frame) never changes
                ptk = ptI[:, 0, :] if k == 2 else ptP[:, k, :]
                compute_k(ptI[:, 1, :], ptk, scn, k)

        # single 4.6 KB mask store for the host merge
        nc.sync.dma_start(ym_ap, mask_sb[:, :])


@functools.cache
def _build_nc():
    import concourse.bacc as bacc
    import concourse.tile as tile
    from concourse import mybir

    nc = bacc.Bacc(
        "TRN2",
        target_bir_lowering=False,
        debug=False,
        enable_asserts=False,
        num_devices=N_CORES,
    )
    x = nc.dram_tensor(
        "x", [TOK_PER_CORE, HIDDEN], mybir.dt.float32, kind="ExternalInput"
    ).ap()
    ys = [
        nc.dram_tensor(
            f"ys{c}", [PATCHES, HIDDEN], mybir.dt.float32, kind="ExternalOutput"
        ).ap()
        for c in range(N_YS)
    ]
    ym = nc.dram_tensor(
        "ym", [128, N_MASK_COLS], mybir.dt.float32, kind="ExternalOutput"
    ).ap()
    with tile.TileContext(nc) as tc:
        _kernel_body(tc, ys, ym, x)
    nc.compile()
    return nc


def _in_maps(hs: np.ndarray):
    img = hs[0, IMG_START : IMG_START + IMG_LEN]
    maps = []
    for c in range(N_CORES):
        xc = img[TOK_PER_CORE * c : TOK_PER_CORE * (c + 1)]
        maps.append({"x": np.ascontiguousarray(xc)})
    return maps


def _host_runt(img: np.ndarray) -> np.ndarray:
    """Blend for patches 192-195 (the %16 runt the device skips): numpy."""
    iv = img.reshape(NUM_FRAMES // INTERVAL, INTERVAL, PATCHES, HIDDEN)
    runt = iv[:, :, 192:PATCHES, :]
    itok = runt[:, I_POS : I_POS + 1]
    d = itok.astype(np.float64) - runt.astype(np.float64)
    mask = np.abs(d).sum(-1) < THRESHOLD
    return np.where(mask[..., None], itok, runt).astype(np.float32)


def _mask_cells(ym: np.ndarray):
    """Decode the [128, 9] mask tile.

    Yields (iv, k, ys_idx, patches, part_rows): masked patch numbers and the
    partition rows they came from, per (interval, P-frame) scatter tensor.
    """
    for sc in range(3):
        for k in range(3):
            col = ym[:, sc * 3 + k]
            if sc < 2:  # A0 / A1: partition p == patch p
                sel = np.nonzero(col[0:128] > 0.5)[0]
                yield sc, k, sc * 6 + k * 2 + 0, sel, sel
            else:  # packed B: rows [0:64) iv0, [64:128) iv1; patch 128+r
                for ivx in range(IVS_PER_CORE):
                    r = np.nonzero(col[ivx * 64 : ivx * 64 + 64] > 0.5)[0]
                    yield ivx, k, ivx * 6 + k * 2 + 1, 128 + r, ivx * 64 + r


def kernel(hidden_states: np.ndarray) -> np.ndarray:
    from concourse.bass_utils import run_bass_kernel_spmd

    hs = np.asarray(hidden_states, dtype=np.float32)
    assert hs.shape == (1, SEQ_LEN, HIDDEN), hs.shape
    nc = _build_nc()
    res = run_bass_kernel_spmd(nc, _in_maps(hs), list(range(N_CORES)))
    out = hs.copy()
    img_flat = out[0, IMG_START : IMG_START + IMG_LEN]  # [12544, 4096] view
    for c in range(N_CORES):
        ym = np.asarray(res.results[c]["ym"])   # [128, 9] 0/1 mask columns
        for ivx, k, ys_idx, patches, _rows in _mask_cells(ym):
            if patches.size:
                ys = np.asarray(res.results[c][f"ys{ys_idx}"])
                r = (ivx * INTERVAL + k) * PATCHES
                img_flat[TOK_PER_CORE * c + r + patches] = ys[patches]
    img = hs[0, IMG_START : IMG_START + IMG_LEN]
    outv = img_flat.reshape(NUM_FRAMES, PATCHES, HIDDEN)
    outv[:, 192:PATCHES, :] = _host_runt(img).reshape(NUM_FRAMES, 4, HIDDEN)
    return out
